# revision 7
# baseline (speedup 1.0000x reference)
"""Trainium2 Bass kernel for Aimv2VisionEmbeddings (patch-embed GEMM + RMSNorm
+ ragged 2D sincos positional embedding), data-parallel over 8 NeuronCores.

Contract: kernel(**inputs) takes the FULL unsharded inputs and returns the
FULL [16, 4096, 1024] float32 output. Internally:
  - batch is sharded 2-per-core across 8 cores,
  - hidden_states is host-cast to bf16 and K-padded 588->640 so the device
    can DMA-transpose (xbar needs 2-byte dtype, 128-col multiples),
  - patch_weight is host-transposed to [K, H] bf16 (tiny),
  - per-row (j, i) position indices ship as a small pre-masked f32 input
    (DVE has no integer mod; the indices are derived from the 16x2
    spatial_shapes, which is marshalling, not compute).

Device program per core (rows = 2*4096):
  GEMM on TensorE: out[r,:] accumulated in PSUM over 5 K-chunks of 128,
  per 128-row tile (lhsT = DMA-transposed X chunk, rhs = W chunk).
  RMS stats on ScalarE (Square + free-dim accum), rstd via Sqrt + DVE
  reciprocal. Positional angles on VectorE; the scalar-engine Sin LUT only
  accepts [-pi, pi], so low omega columns (largest angles) get a
  k = trunc(t/2pi) subtract plus an add_range_wrap custom-DVE op; high
  columns use Sin(t) / Sin(pi/2 - t) directly. Final (x * rstd) + pos is a
  single fused scalar_tensor_tensor pass straight out of PSUM.
"""

import numpy as np
import ml_dtypes

import concourse.bass as bass
import concourse.bacc as bacc
import concourse.mybir as mybir
from concourse import tile
from concourse.bass_utils import run_bass_kernel_spmd

AF = mybir.ActivationFunctionType
ALU = mybir.AluOpType
DT = mybir.dt

B, N, D, H = 16, 4096, 588, 1024
NCORES = 8
LB = B // NCORES          # local batches per core
KP, NK = 640, 5           # zero-padded contraction dim, 5 chunks of 128
POS_DIM = H // 4          # 256
EPS = 1e-6
TEMP = 10000.0
PI = float(np.pi)
TWO_PI = float(2.0 * np.pi)


def redzone_width(idx_max):
    """Columns d < rz need range reduction: idx_max * 10^(-d/64) > pi."""
    if idx_max <= np.pi:
        return 0
    rz = int(np.ceil(64.0 * np.log10(idx_max / np.pi)))
    rz = min(POS_DIM, (rz + 15) // 16 * 16)
    return rz


def build(rows_per_b=N, rb=2048, with_bias=False, with_rmsw=False, rz=96,
          psum_bufs=3, xt_bufs=2, work_bufs=3):
    """Build the per-core bass program. rows_per_b/rb are shrinkable for sim."""
    rows = LB * rows_per_b
    rb = min(rb, rows_per_b)
    assert rows_per_b % rb == 0 and rb % 128 == 0
    assert 0 < rz <= POS_DIM

    nc = bacc.Bacc("TRN2", target_bir_lowering=False, debug=False)
    x_d = nc.declare_dram_parameter("x", [rows, KP], DT.bfloat16, isOutput=False)
    w_d = nc.declare_dram_parameter("w", [KP, H], DT.bfloat16, isOutput=False)
    ij_d = nc.declare_dram_parameter("ij", [rows, 2], DT.float32, isOutput=False)
    om_d = nc.declare_dram_parameter("om", [128, POS_DIM], DT.float32, isOutput=False)
    if with_bias:
        bias_d = nc.declare_dram_parameter("bias", [128, H], DT.float32, isOutput=False)
    if with_rmsw:
        rw_d = nc.declare_dram_parameter("rw", [128, H], DT.float32, isOutput=False)
    out_d = nc.declare_dram_parameter("out", [rows, H], DT.float32, isOutput=True)

    with tile.TileContext(nc) as tc:
        with (
            tc.tile_pool(name="const", bufs=1) as cpool,
            tc.tile_pool(name="xt", bufs=xt_bufs) as xpool,
            tc.tile_pool(name="work", bufs=work_bufs) as wpool,
            tc.tile_pool(name="psum", bufs=psum_bufs, space=bass.MemorySpace.PSUM) as ppool,
        ):
            wt = cpool.tile([128, NK, H], DT.bfloat16)
            nc.sync.dma_start(wt[:], w_d.rearrange("(k p) h -> p k h", p=128))
            om = cpool.tile([128, POS_DIM], DT.float32)
            nc.sync.dma_start(om[:], om_d[:])
            pio2 = cpool.tile([128, 1], DT.float32)
            nc.vector.memset(pio2[:], PI / 2)
            epst = cpool.tile([128, 1], DT.float32)
            nc.vector.memset(epst[:], EPS)
            if with_bias:
                biast = cpool.tile([128, H], DT.float32)
                nc.sync.dma_start(biast[:], bias_d[:])
            if with_rmsw:
                rwt = cpool.tile([128, H], DT.float32)
                nc.sync.dma_start(rwt[:], rw_d[:])

            n_blocks = rows // rb
            tiles_per_blk = rb // 128
            for blk in range(n_blocks):
                r0 = blk * rb
                xts = []
                for k in range(NK):
                    xt_k = xpool.tile([128, rb], DT.bfloat16, tag=f"xt{k}")
                    nc.sync.dma_start_transpose(
                        xt_k[:], x_d[r0:r0 + rb, k * 128:(k + 1) * 128]
                    )
                    xts.append(xt_k)
                ijb = xpool.tile([128, tiles_per_blk, 2], DT.float32, tag="ijb")
                nc.sync.dma_start(
                    ijb[:], ij_d[r0:r0 + rb, :].rearrange("(t p) c -> p t c", p=128)
                )

                for it in range(tiles_per_blk):
                    row0 = r0 + it * 128

                    xacc = ppool.tile([128, H], DT.float32, tag="xacc")
                    for half in range(2):
                        for k in range(NK):
                            nc.tensor.matmul(
                                xacc[:, half * 512:(half + 1) * 512],
                                xts[k][:, it * 128:(it + 1) * 128],
                                wt[:, k, half * 512:(half + 1) * 512],
                                start=(k == 0),
                                stop=(k == NK - 1),
                            )

                    if with_bias:
                        xsrc = wpool.tile([128, H], DT.float32, tag="xb")
                        nc.vector.tensor_add(xsrc[:], xacc[:], biast[:])
                    else:
                        xsrc = xacc

                    jm = ijb[:, it, 0:1]
                    im = ijb[:, it, 1:2]

                    # raw angles t = idx * omega, halves [h | w]
                    ang = wpool.tile([128, 2, POS_DIM], DT.float32, tag="ang")
                    nc.vector.tensor_scalar(ang[:, 0, :], om[:], jm, None, ALU.mult)
                    nc.vector.tensor_scalar(ang[:, 1, :], om[:], im, None, ALU.mult)

                    # range reduction for the first rz columns of each half:
                    # k = int(t/2pi) (trunc or rne - either works), u = t-2pi*k
                    # lands in (-pi, 2pi); add_range_wrap folds into (-pi, pi].
                    angz = ang[:, :, 0:rz]
                    ki = wpool.tile([128, 2, rz], DT.int32, tag="ki")
                    nc.vector.tensor_scalar(ki[:], angz, 1.0 / TWO_PI, None, ALU.mult)
                    red = wpool.tile([128, 2, rz], DT.float32, tag="red")
                    nc.vector.scalar_tensor_tensor(
                        red[:], ki[:], -TWO_PI, angz, ALU.mult, ALU.add
                    )
                    us = wpool.tile([128, 2, rz], DT.float32, tag="us")
                    nc.vector.add_range_wrap(us[:], red[:], 0.0, PI, TWO_PI)
                    uc = wpool.tile([128, 2, rz], DT.float32, tag="uc")
                    nc.vector.add_range_wrap(uc[:], red[:], PI / 2, PI, TWO_PI)

                    # pos layout: [sin_h | cos_h | sin_w | cos_w] each POS_DIM
                    pos = wpool.tile([128, 4, POS_DIM], DT.float32, tag="pos")
                    sin_v = pos.rearrange("p (a b) d -> p a b d", a=2)
                    # sin halves: pos[:, 0, :] (h) and pos[:, 2, :] (w)
                    nc.scalar.activation(sin_v[:, :, 0, 0:rz], us[:], AF.Sin)
                    # cos halves: pos[:, 1, :] (h) and pos[:, 3, :] (w)
                    nc.scalar.activation(sin_v[:, :, 1, 0:rz], uc[:], AF.Sin)
                    if rz < POS_DIM:
                        nc.scalar.activation(
                            sin_v[:, :, 0, rz:POS_DIM], ang[:, :, rz:POS_DIM], AF.Sin
                        )
                        # cos(t) = sin(pi/2 - t), valid while t <= 3pi/2
                        nc.scalar.activation(
                            sin_v[:, :, 1, rz:POS_DIM], ang[:, :, rz:POS_DIM],
                            AF.Sin, bias=pio2[:], scale=-1.0,
                        )

                    # --- RMS stats ---
                    sqd = wpool.tile([128, H], DT.float32, tag="sqd")
                    ssq = wpool.tile([128, 1], DT.float32, tag="ssq")
                    nc.scalar.activation(sqd[:], xsrc[:], AF.Square, accum_out=ssq[:])
                    std = wpool.tile([128, 1], DT.float32, tag="std")
                    nc.scalar.activation(std[:], ssq[:], AF.Sqrt, scale=1.0 / H, bias=epst[:])
                    rstd = wpool.tile([128, 1], DT.float32, tag="rstd")
                    nc.vector.reciprocal(rstd[:], std[:])

                    outt = wpool.tile([128, H], DT.float32, tag="outt")
                    posf = pos.rearrange("p a d -> p (a d)")
                    if with_rmsw:
                        xn = wpool.tile([128, H], DT.float32, tag="xn")
                        nc.vector.tensor_scalar(xn[:], xsrc[:], rstd[:], None, ALU.mult)
                        nc.vector.tensor_mul(xn[:], xn[:], rwt[:])
                        nc.vector.tensor_add(outt[:], xn[:], posf)
                    else:
                        nc.vector.scalar_tensor_tensor(
                            outt[:], xsrc[:], rstd[:], posf, ALU.mult, ALU.add
                        )
                    nc.scalar.dma_start(out_d[row0:row0 + 128, :], outt[:])

    nc.compile()
    return nc


def make_inputs(hidden_states, spatial_shapes, patch_weight, patch_bias,
                rms_weight, rows_per_b=N):
    """Host-side marshalling: shard + cast + pad. Returns (in_maps, meta)."""
    hs = np.asarray(hidden_states, dtype=np.float32)
    ss = np.asarray(spatial_shapes)
    pw = np.asarray(patch_weight, dtype=np.float32).reshape(H, D)
    pb = np.asarray(patch_bias, dtype=np.float32)
    rw = np.asarray(rms_weight, dtype=np.float32)
    with_bias = bool(np.any(pb != 0.0))
    with_rmsw = bool(np.any(rw != 1.0))

    bf16 = ml_dtypes.bfloat16
    hsv = hs[:, :rows_per_b, :]          # [B, rows_per_b, D]
    xp = np.zeros((B * rows_per_b, KP), dtype=bf16)
    xp[:, :D] = hsv.reshape(B * rows_per_b, D).astype(bf16)
    wp = np.zeros((KP, H), dtype=bf16)
    wp[:D, :] = pw.T.astype(bf16)

    om = (1.0 / (TEMP ** (np.arange(POS_DIM, dtype=np.float32) / POS_DIM))).astype(np.float32)
    om128 = np.ascontiguousarray(np.broadcast_to(om, (128, POS_DIM)))

    # per-row (j, i) indices, pre-masked (invalid rows -> 0), as f32
    n = np.arange(rows_per_b, dtype=np.int64)[None, :]       # [1, R]
    hcol = ss[:, 0:1].astype(np.int64)
    wcol = ss[:, 1:2].astype(np.int64)
    valid = n < hcol * wcol
    jv = np.where(valid, n % wcol, 0).astype(np.float32)     # [B, R]
    iv = np.where(valid, n // wcol, 0).astype(np.float32)
    ij = np.stack([jv, iv], axis=-1).reshape(B * rows_per_b, 2)
    ij = np.ascontiguousarray(ij, dtype=np.float32)

    idx_max = float(max(jv.max(), iv.max(), 1.0))
    rz = max(16, redzone_width(idx_max))

    rows = LB * rows_per_b
    in_maps = []
    for c in range(NCORES):
        m = {
            "x": xp[c * rows:(c + 1) * rows],
            "w": wp,
            "ij": ij[c * rows:(c + 1) * rows],
            "om": om128,
        }
        if with_bias:
            m["bias"] = np.ascontiguousarray(np.broadcast_to(pb, (128, H)))
        if with_rmsw:
            m["rw"] = np.ascontiguousarray(np.broadcast_to(rw, (128, H)))
        in_maps.append(m)
    return in_maps, with_bias, with_rmsw, rz


_BUILD_CACHE = {}


def kernel(hidden_states, spatial_shapes, patch_weight, patch_bias, rms_weight,
           _trace=False):
    in_maps, with_bias, with_rmsw, rz = make_inputs(
        hidden_states, spatial_shapes, patch_weight, patch_bias, rms_weight
    )
    key = (with_bias, with_rmsw, rz)
    if key not in _BUILD_CACHE:
        _BUILD_CACHE[key] = build(with_bias=with_bias, with_rmsw=with_rmsw, rz=rz)
    nc = _BUILD_CACHE[key]
    res = run_bass_kernel_spmd(nc, in_maps, list(range(NCORES)), trace=_trace)
    out = np.concatenate([r["out"] for r in res.results], axis=0)
    out = out.reshape(B, N, H).astype(np.float32, copy=False)
    if _trace:
        kernel.last_results = res
    return out


# revision 12
# speedup vs baseline: 1.4308x; 1.4308x over previous
"""Trainium2 Bass kernel for Aimv2VisionEmbeddings (patch-embed GEMM + RMSNorm
+ ragged 2D sincos positional embedding), data-parallel over 8 NeuronCores.

Contract: kernel(**inputs) takes the FULL unsharded inputs and returns the
FULL [16, 4096, 1024] float32 output. Internally:
  - batch is sharded 2-per-core across 8 cores,
  - hidden_states is host-cast to bf16 and K-padded 588->640 so the device
    can DMA-transpose (xbar needs 2-byte dtype, 128-col multiples),
  - patch_weight is host-transposed to [K, H] bf16 (tiny),
  - per-row (j, i) position indices ship as a small pre-masked f32 input
    (DVE has no integer mod; the indices are derived from the 16x2
    spatial_shapes, which is marshalling, not compute).

Device program per core (rows = 2*4096):
  GEMM on TensorE: out[r,:] accumulated in PSUM over 5 K-chunks of 128,
  per 128-row tile (lhsT = DMA-transposed X chunk, rhs = W chunk).
  RMS stats on ScalarE (Square + free-dim accum), rstd via Sqrt + DVE
  reciprocal. Positional angles on VectorE; the scalar-engine Sin LUT only
  accepts [-pi, pi], so low omega columns (largest angles) get a
  k = trunc(t/2pi) subtract plus an add_range_wrap custom-DVE op; high
  columns use Sin(t) / Sin(pi/2 - t) directly. Final (x * rstd) + pos is a
  single fused scalar_tensor_tensor pass straight out of PSUM.
"""

import numpy as np
import ml_dtypes

import concourse.bass as bass
import concourse.bacc as bacc
import concourse.mybir as mybir
from concourse import tile
from concourse.bass_utils import run_bass_kernel_spmd

AF = mybir.ActivationFunctionType
ALU = mybir.AluOpType
DT = mybir.dt

B, N, D, H = 16, 4096, 588, 1024
NCORES = 8
LB = B // NCORES          # local batches per core
KP, NK = 640, 5           # zero-padded contraction dim, 5 chunks of 128
POS_DIM = H // 4          # 256
EPS = 1e-6
TEMP = 10000.0
PI = float(np.pi)
TWO_PI = float(2.0 * np.pi)


def redzone_width(idx_max):
    """Columns d < rz need range reduction: idx_max * 10^(-d/64) > pi."""
    if idx_max <= np.pi:
        return 0
    rz = int(np.ceil(64.0 * np.log10(idx_max / np.pi)))
    rz = min(POS_DIM, (rz + 15) // 16 * 16)
    return rz


QUAKE_C = 0x5F3759DF


def build(rows_per_b=N, rb=2048, with_bias=False, with_rmsw=False, rz=96,
          psum_bufs=4, xt_bufs=2, work_bufs=4, grp=3, out_bf16=True):
    """Build the per-core bass program. rows_per_b/rb are shrinkable for sim."""
    rows = LB * rows_per_b
    rb = min(rb, rows_per_b)
    assert rows_per_b % rb == 0 and rb % 128 == 0
    assert 0 < rz <= POS_DIM
    out_dt = DT.bfloat16 if out_bf16 else DT.float32

    nc = bacc.Bacc("TRN2", target_bir_lowering=False, debug=False)
    x_d = nc.declare_dram_parameter("x", [rows, KP], DT.bfloat16, isOutput=False)
    w_d = nc.declare_dram_parameter("w", [KP, H], DT.bfloat16, isOutput=False)
    ij_d = nc.declare_dram_parameter("ij", [rows, 2], DT.float32, isOutput=False)
    om_d = nc.declare_dram_parameter("om", [128, POS_DIM], DT.float32, isOutput=False)
    if with_bias:
        bias_d = nc.declare_dram_parameter("bias", [128, H], DT.float32, isOutput=False)
    if with_rmsw:
        rw_d = nc.declare_dram_parameter("rw", [128, H], DT.float32, isOutput=False)
    out_d = nc.declare_dram_parameter("out", [rows, H], out_dt, isOutput=True)

    with tile.TileContext(nc) as tc:
        with (
            tc.tile_pool(name="const", bufs=1) as cpool,
            tc.tile_pool(name="xt", bufs=xt_bufs) as xpool,
            tc.tile_pool(name="work", bufs=work_bufs) as wpool,
            tc.tile_pool(name="psum", bufs=psum_bufs, space=bass.MemorySpace.PSUM) as ppool,
        ):
            wt = cpool.tile([128, NK, H], DT.bfloat16)
            nc.sync.dma_start(wt[:], w_d.rearrange("(k p) h -> p k h", p=128))
            om = cpool.tile([128, POS_DIM], DT.float32)
            nc.sync.dma_start(om[:], om_d[:])
            pio2 = cpool.tile([128, 1], DT.float32)
            nc.vector.memset(pio2[:], PI / 2)
            cq = cpool.tile([128, grp], DT.int32)
            nc.vector.memset(cq[:], QUAKE_C)
            if with_bias:
                biast = cpool.tile([128, H], DT.float32)
                nc.sync.dma_start(biast[:], bias_d[:])
            if with_rmsw:
                rwt = cpool.tile([128, H], DT.float32)
                nc.sync.dma_start(rwt[:], rw_d[:])

            n_blocks = rows // rb
            tiles_per_blk = rb // 128
            for blk in range(n_blocks):
                r0 = blk * rb
                xts = []
                for k in range(NK):
                    xt_k = xpool.tile([128, rb], DT.bfloat16, tag=f"xt{k}")
                    nc.sync.dma_start_transpose(
                        xt_k[:], x_d[r0:r0 + rb, k * 128:(k + 1) * 128]
                    )
                    xts.append(xt_k)
                ijb = xpool.tile([128, tiles_per_blk, 2], DT.float32, tag="ijb")
                nc.sync.dma_start(
                    ijb[:], ij_d[r0:r0 + rb, :].rearrange("(t p) c -> p t c", p=128)
                )

                it = 0
                while it < tiles_per_blk:
                    g = min(grp, tiles_per_blk - it)
                    ssqg = wpool.tile([128, grp], DT.float32, tag="ssqg")
                    xsrcs, poss = [], []
                    for gi in range(g):
                        t = it + gi
                        xacc = ppool.tile([128, H], DT.float32, tag="xacc")
                        for half in range(2):
                            for k in range(NK):
                                nc.tensor.matmul(
                                    xacc[:, half * 512:(half + 1) * 512],
                                    xts[k][:, t * 128:(t + 1) * 128],
                                    wt[:, k, half * 512:(half + 1) * 512],
                                    start=(k == 0),
                                    stop=(k == NK - 1),
                                )

                        if with_bias:
                            xsrc = wpool.tile([128, H], DT.float32, tag="xb")
                            nc.vector.tensor_add(xsrc[:], xacc[:], biast[:])
                        else:
                            xsrc = xacc
                        xsrcs.append(xsrc)

                        jm = ijb[:, t, 0:1]
                        im = ijb[:, t, 1:2]

                        # raw angles t = idx * omega, halves [h | w]
                        ang = wpool.tile([128, 2, POS_DIM], DT.float32, tag="ang")
                        nc.vector.tensor_scalar(ang[:, 0, :], om[:], jm, None, ALU.mult)
                        nc.vector.tensor_scalar(ang[:, 1, :], om[:], im, None, ALU.mult)

                        # range reduction for the first rz columns of each half:
                        # kq = int(t/2pi) (trunc or rne - either works),
                        # u = t-2pi*kq lands in (-pi, 2pi); add_range_wrap
                        # folds sin/cos args into (-pi, pi].
                        angz = ang[:, :, 0:rz]
                        ki = wpool.tile([128, 2, rz], DT.int32, tag="ki")
                        nc.vector.tensor_scalar(ki[:], angz, 1.0 / TWO_PI, None, ALU.mult)
                        red = wpool.tile([128, 2, rz], DT.float32, tag="red")
                        nc.vector.scalar_tensor_tensor(
                            red[:], ki[:], -TWO_PI, angz, ALU.mult, ALU.add
                        )
                        # usuc[:, 0] = sin args, usuc[:, 1] = cos args
                        usuc = wpool.tile([128, 2, 2, rz], DT.float32, tag="usuc")
                        nc.vector.add_range_wrap(usuc[:, 0], red[:], 0.0, PI, TWO_PI)
                        nc.vector.add_range_wrap(usuc[:, 1], red[:], PI / 2, PI, TWO_PI)

                        # pos layout: [sin_h | cos_h | sin_w | cos_w] each POS_DIM
                        pos = wpool.tile([128, 4, POS_DIM], DT.float32, tag="pos")
                        poss.append(pos)
                        # view as [p, sincos, half, d]: segment a = half*2 + sincos
                        sc_v = pos.rearrange("p (b s) d -> p s b d", b=2)
                        nc.scalar.activation(sc_v[:, :, :, 0:rz], usuc[:], AF.Sin)
                        sin_v = pos.rearrange("p (a b) d -> p a b d", a=2)
                        if rz < POS_DIM:
                            nc.scalar.activation(
                                sin_v[:, :, 0, rz:POS_DIM], ang[:, :, rz:POS_DIM], AF.Sin
                            )
                            # cos(t) = sin(pi/2 - t), valid while t <= 3pi/2
                            nc.scalar.activation(
                                sin_v[:, :, 1, rz:POS_DIM], ang[:, :, rz:POS_DIM],
                                AF.Sin, bias=pio2[:], scale=-1.0,
                            )

                        # sum of squares for this tile -> ssqg[:, gi]
                        sqd = wpool.tile([128, H], DT.float32, tag="sqd")
                        nc.scalar.activation(
                            sqd[:], xsrc[:], AF.Square, accum_out=ssqg[:, gi:gi + 1]
                        )

                    # rstd = rsqrt(ssq/H + eps) for the whole group on DVE
                    # (bitcast seed + 2 Newton steps; avoids the Sqrt ACT
                    # table, so ScalarE never swaps LUT sets).
                    gs = slice(0, g)
                    vq = wpool.tile([128, grp], DT.float32, tag="vq")
                    nc.vector.tensor_scalar(vq[:, gs], ssqg[:, gs], 1.0 / H, EPS, ALU.mult, ALU.add)
                    ish = wpool.tile([128, grp], DT.int32, tag="ish")
                    nc.vector.tensor_scalar(
                        ish[:, gs], vq[:, gs].bitcast(DT.int32), 1, None, ALU.arith_shift_right
                    )
                    y0 = wpool.tile([128, grp], DT.int32, tag="y0")
                    nc.vector.tensor_sub(y0[:, gs], cq[:, gs], ish[:, gs])
                    y0f = y0[:, gs].bitcast(DT.float32)
                    qa = wpool.tile([128, grp], DT.float32, tag="qa")
                    nc.vector.tensor_mul(qa[:, gs], y0f, y0f)
                    nc.vector.tensor_mul(qa[:, gs], qa[:, gs], vq[:, gs])
                    nc.vector.tensor_scalar(qa[:, gs], qa[:, gs], -0.5, 1.5, ALU.mult, ALU.add)
                    qy = wpool.tile([128, grp], DT.float32, tag="qy")
                    nc.vector.tensor_mul(qy[:, gs], y0f, qa[:, gs])
                    qb = wpool.tile([128, grp], DT.float32, tag="qb")
                    nc.vector.tensor_mul(qb[:, gs], qy[:, gs], qy[:, gs])
                    nc.vector.tensor_mul(qb[:, gs], qb[:, gs], vq[:, gs])
                    nc.vector.tensor_scalar(qb[:, gs], qb[:, gs], -0.5, 1.5, ALU.mult, ALU.add)
                    rstdg = wpool.tile([128, grp], DT.float32, tag="rstdg")
                    nc.vector.tensor_mul(rstdg[:, gs], qy[:, gs], qb[:, gs])

                    for gi in range(g):
                        t = it + gi
                        row0 = r0 + t * 128
                        rs = rstdg[:, gi:gi + 1]
                        outt = wpool.tile([128, H], out_dt, tag="outt")
                        posf = poss[gi].rearrange("p a d -> p (a d)")
                        if with_rmsw:
                            xn = wpool.tile([128, H], DT.float32, tag="xn")
                            nc.vector.tensor_scalar(xn[:], xsrcs[gi][:], rs, None, ALU.mult)
                            nc.vector.tensor_mul(xn[:], xn[:], rwt[:])
                            nc.vector.tensor_add(outt[:], xn[:], posf)
                        else:
                            nc.vector.scalar_tensor_tensor(
                                outt[:], xsrcs[gi][:], rs, posf, ALU.mult, ALU.add
                            )
                        nc.scalar.dma_start(out_d[row0:row0 + 128, :], outt[:])
                    it += g

    nc.compile()
    return nc


def make_inputs(hidden_states, spatial_shapes, patch_weight, patch_bias,
                rms_weight, rows_per_b=N):
    """Host-side marshalling: shard + cast + pad. Returns (in_maps, meta)."""
    hs = np.asarray(hidden_states, dtype=np.float32)
    ss = np.asarray(spatial_shapes)
    pw = np.asarray(patch_weight, dtype=np.float32).reshape(H, D)
    pb = np.asarray(patch_bias, dtype=np.float32)
    rw = np.asarray(rms_weight, dtype=np.float32)
    with_bias = bool(np.any(pb != 0.0))
    with_rmsw = bool(np.any(rw != 1.0))

    bf16 = ml_dtypes.bfloat16
    hsv = hs[:, :rows_per_b, :]          # [B, rows_per_b, D]
    xp = np.zeros((B * rows_per_b, KP), dtype=bf16)
    xp[:, :D] = hsv.reshape(B * rows_per_b, D).astype(bf16)
    wp = np.zeros((KP, H), dtype=bf16)
    wp[:D, :] = pw.T.astype(bf16)

    om = (1.0 / (TEMP ** (np.arange(POS_DIM, dtype=np.float32) / POS_DIM))).astype(np.float32)
    om128 = np.ascontiguousarray(np.broadcast_to(om, (128, POS_DIM)))

    # per-row (j, i) indices, pre-masked (invalid rows -> 0), as f32
    n = np.arange(rows_per_b, dtype=np.int64)[None, :]       # [1, R]
    hcol = ss[:, 0:1].astype(np.int64)
    wcol = ss[:, 1:2].astype(np.int64)
    valid = n < hcol * wcol
    jv = np.where(valid, n % wcol, 0).astype(np.float32)     # [B, R]
    iv = np.where(valid, n // wcol, 0).astype(np.float32)
    ij = np.stack([jv, iv], axis=-1).reshape(B * rows_per_b, 2)
    ij = np.ascontiguousarray(ij, dtype=np.float32)

    idx_max = float(max(jv.max(), iv.max(), 1.0))
    rz = max(16, redzone_width(idx_max))

    rows = LB * rows_per_b
    in_maps = []
    for c in range(NCORES):
        m = {
            "x": xp[c * rows:(c + 1) * rows],
            "w": wp,
            "ij": ij[c * rows:(c + 1) * rows],
            "om": om128,
        }
        if with_bias:
            m["bias"] = np.ascontiguousarray(np.broadcast_to(pb, (128, H)))
        if with_rmsw:
            m["rw"] = np.ascontiguousarray(np.broadcast_to(rw, (128, H)))
        in_maps.append(m)
    return in_maps, with_bias, with_rmsw, rz


_BUILD_CACHE = {}


def kernel(hidden_states, spatial_shapes, patch_weight, patch_bias, rms_weight,
           _trace=False):
    in_maps, with_bias, with_rmsw, rz = make_inputs(
        hidden_states, spatial_shapes, patch_weight, patch_bias, rms_weight
    )
    key = (with_bias, with_rmsw, rz)
    if key not in _BUILD_CACHE:
        _BUILD_CACHE[key] = build(with_bias=with_bias, with_rmsw=with_rmsw, rz=rz)
    nc = _BUILD_CACHE[key]
    res = run_bass_kernel_spmd(nc, in_maps, list(range(NCORES)), trace=_trace)
    out = np.concatenate([r["out"] for r in res.results], axis=0)
    out = out.reshape(B, N, H).astype(np.float32, copy=False)
    if _trace:
        kernel.last_results = res
    return out
